# revision 17
# baseline (speedup 1.0000x reference)
"""Trainium2 Bass kernel for LorentzInvariantPositionalEncoding.

Reference computation (B=32, N=512, D=512):
  out[b,i,d] = x[b,i,d] + pe[i,d]
  arg[b,i,j] = sum_{k=1..3} (xc[b,i,k]-xc[b,j,k])^2 - (xc[b,i,0]-xc[b,j,0])^2
  ld[b,i,j]  = sqrt(relu(arg))

Strategy: pure data parallel over batch, 4 batches per core on 8 cores.
The kernel is HBM-bandwidth bound, so all bulk I/O is bf16 (the 2e-2
rel-err budget dwarfs bf16's ~0.4% worst-case): x is cast to bf16 on the
host, pe is baked into the NEFF as a pre-arranged bf16 inline constant,
and out/ld are stored as bf16 and upcast on the host.  That cuts per-core
HBM traffic from ~13 MB to ~6.6 MB.

Per batch the Minkowski pairwise matrix comes from the Gram trick:
  arg = q_i + q_j - 2 * <c_i, eta*c_j>,   q_i = sum_k eta_k c_ik^2
as one K=16 float32r matmul per 128-row output chunk.  A Dekker-style
hi/lo split of c and q recovers fp32-level accuracy (fp32r rounds
operands to ~12-bit mantissa; the split pieces are each 12-bit exact).
Because the split pieces are 12-bit exact, the PE transposes that move
the operands to K-layout can themselves run in fp32r (1 cycle/row vs 4)
losslessly.  Operand assembly is done for all 4 batches at once in 12
wide DVE ops.  relu on DVE (PSUM f32 -> SBUF bf16), sqrt on ACT in bf16,
x+pe add split DVE/GpSimd, all big DMAs on the HWDGE rings.
"""

from contextlib import ExitStack

import numpy as np
import ml_dtypes

import concourse.bass as bass
import concourse.tile as tile
from concourse import bacc, mybir
from concourse.bass_utils import run_bass_kernel_spmd

B, N, D = 32, 512, 512
MAX_LEN = 5000
NCORES = 8
BP = B // NCORES  # batches per core
P = 128
NCH = N // P  # 4 partition chunks of the i dimension
K = 16

_F32 = mybir.dt.float32
_F32R = mybir.dt.float32r
_BF16 = mybir.dt.bfloat16
_BFNP = ml_dtypes.bfloat16

_cached_nc = None


def _make_pe_bf16():
    # Deterministic sinusoidal PE (identical formula to the reference),
    # first N rows only, pre-arranged so partition p holds rows 4p+n.
    position = np.arange(N, dtype=np.float32)[:, None]
    div_term = np.exp(
        np.arange(0, D, 2, dtype=np.float32) * (-np.log(10000.0) / D)
    )
    pe = np.zeros((N, D), dtype=np.float32)
    pe[:, 0::2] = np.sin(position * div_term)
    pe[:, 1::2] = np.cos(position * div_term)
    return pe.reshape(P, NCH * D).astype(_BFNP)


def _build():
    global _cached_nc
    if _cached_nc is not None:
        return _cached_nc

    nc = bacc.Bacc("TRN2", target_bir_lowering=False, debug=False, num_devices=NCORES)

    x_in = nc.dram_tensor("x", [BP, N, D], _BF16, kind="ExternalInput")
    # coords come host-prearranged: xc[p, (b, q, k)] = x_coords[b, 4p+q, k],
    # so the load is one straight 128x256B DMA instead of 512 64B descriptors
    # (which cost ~4.3 us of startup latency on the lorentz critical path).
    xc_in = nc.dram_tensor("xc", [P, BP * NCH * 4], _F32, kind="ExternalInput")
    out_o = nc.dram_tensor("out", [BP, N, D], _BF16, kind="ExternalOutput")
    ld_o = nc.dram_tensor("ld", [BP, N, N], _BF16, kind="ExternalOutput")

    # merged const blob per partition:
    # [eta (BP*NCH*4) | -2*eta (BP*NCH*4) | identity (128)]
    eta = np.array([-1.0, 1.0, 1.0, 1.0], np.float32)
    ew = BP * NCH * 4  # 64
    cst_np = np.concatenate(
        [
            np.tile(eta, (P, BP * NCH)),
            np.tile(-2.0 * eta, (P, BP * NCH)),
            np.eye(P, dtype=np.float32),
            np.ones((P, 2 * BP * NCH), np.float32),
        ],
        axis=1,
    )
    cst_in = nc.inline_tensor(cst_np, "cst")
    pe_in = nc.inline_tensor(_make_pe_bf16(), "peb")

    with tile.TileContext(nc) as tc, ExitStack() as ctx:
        cpool = ctx.enter_context(tc.tile_pool(name="const", bufs=1))
        xpool = ctx.enter_context(tc.tile_pool(name="x", bufs=4))
        ldpool = ctx.enter_context(tc.tile_pool(name="ld", bufs=4))
        copool = ctx.enter_context(tc.tile_pool(name="coords", bufs=1))
        mpool = ctx.enter_context(tc.tile_pool(name="mats", bufs=4))
        parg = ctx.enter_context(tc.tile_pool(name="parg", bufs=4, space="PSUM"))
        ptp = ctx.enter_context(tc.tile_pool(name="ptp", bufs=2, space="PSUM"))

        # Dummy sqrt on a memset scratch: pulls the one-time ACT table load
        # (sqrt_and_others, which also contains Copy for the operand copies)
        # to the very start, overlapping the initial DMA latency.
        scr = cpool.tile([P, 2], _F32)
        nc.vector.memset(scr[:], 1.0)
        nc.scalar.sqrt(scr[:], scr[:])
        nc.scalar.copy(scr[:], scr[:])

        # --- loads: coords first (they gate the whole lorentz chain), then
        # consts, all on the sync HWDGE ring; pe rides the scalar ring.
        # coords layout: partition p holds rows 4p+q (the contiguous layout).
        ct_all = copool.tile([P, BP * NCH * 4], _F32)
        nc.sync.dma_start(ct_all[:], xc_in[:])
        NG = BP * NCH  # 16 (batch, group) pairs
        cst = cpool.tile([P, 2 * ew + P + NG * 2], _F32)
        nc.sync.dma_start(cst[:], cst_in[:])
        etat = cst[:, 0:ew]
        m2etat = cst[:, ew : 2 * ew]
        # identity re-materialized through DVE so its producer carries an
        # f32r output dtype (the fp32r-matmul verifier requires operand
        # producers to be f32r-rounded writes).
        ident_t = cpool.tile([P, P], _F32R)
        nc.vector.tensor_copy(ident_t[:], cst[:, 2 * ew : 2 * ew + P])
        identr = ident_t[:]
        ones3 = cst[:, 2 * ew + P :].rearrange("p (g c) -> p g c", c=2)

        pe_t = cpool.tile([P, NCH * D], _BF16)
        nc.scalar.dma_start(pe_t[:], pe_in[:])

        # x loads: bf16, partition p holds rows 4p+n -> one contiguous 4 KiB
        # HBM run per partition per batch.
        xts = []
        for b in range(BP):
            xt = xpool.tile([P, NCH * D], _BF16)
            nc.sync.dma_start(
                xt[:].rearrange("p (n d) -> p n d", n=NCH),
                x_in[b].rearrange("(p n) d -> p n d", n=NCH),
            )
            xts.append(xt)

        # ---- lorentz operand assembly, all batches at once ----
        # fp32r matmuls round their operands (~12-bit mantissa), so use a
        # Dekker-style hi/lo split to recover fp32-level accuracy at K=16
        # (matmul cost depends only on output rows, so K=16 is free).
        # Row pairing (lhsT row, rhs row) by k:
        #  k 0-3: (-2e*ch, ch)  4-7: (-2e*ch, cl)  8-11: (-2e*cl, ch)
        #  k 12: (qh, 1)  13: (ql, 1)  14: (1, qh)  15: (1, ql)
        ct3 = ct_all[:].rearrange("p (g k) -> p g k", g=NG)
        m2eta3 = m2etat.rearrange("p (g k) -> p g k", g=NG)

        t1 = copool.tile([P, ew], _F32)
        nc.vector.tensor_mul(t1[:], ct_all[:], etat)
        t2 = copool.tile([P, ew], _F32)
        nc.vector.tensor_mul(t2[:], t1[:], ct_all[:])
        q_pp = copool.tile([P, NG], _F32)
        nc.vector.tensor_reduce(
            q_pp[:],
            t2[:].rearrange("p (g k) -> p g k", g=NG),
            axis=mybir.AxisListType.X,
            op=mybir.AluOpType.add,
        )
        q3 = q_pp[:].rearrange("p (g u) -> p g u", u=1)

        # All assembly outputs are f32r-typed: the fp32r transposes below
        # require every producer of their operands to be an f32r write.
        # ch/qh/products/ones are 12-bit exact; f32r-rounding cl/ql loses
        # ~1 ulp of the low part (error ~2^-25 relative) — negligible.
        am = copool.tile([P, NG * K], _F32)
        a3 = am[:].rearrange("p (g c) -> p g c", g=NG)
        nc.vector.tensor_copy(a3[:, :, 0:4].bitcast(_F32R), ct3)  # ch
        nc.vector.tensor_sub(a3[:, :, 4:8].bitcast(_F32R), ct3, a3[:, :, 0:4])  # cl
        nc.vector.tensor_copy(a3[:, :, 8:12].bitcast(_F32R), a3[:, :, 0:4])
        nc.vector.tensor_copy(a3[:, :, 12:14].bitcast(_F32R), ones3)
        nc.vector.tensor_copy(a3[:, :, 14:15].bitcast(_F32R), q3)  # qh
        nc.vector.tensor_sub(a3[:, :, 15:16].bitcast(_F32R), q3, a3[:, :, 14:15])  # ql

        bm = copool.tile([P, NG * K], _F32)
        b3 = bm[:].rearrange("p (g c) -> p g c", g=NG)
        nc.vector.tensor_mul(b3[:, :, 0:4].bitcast(_F32R), a3[:, :, 0:4], m2eta3)
        nc.vector.tensor_copy(b3[:, :, 4:8].bitcast(_F32R), b3[:, :, 0:4])
        nc.vector.tensor_mul(b3[:, :, 8:12].bitcast(_F32R), a3[:, :, 4:8], m2eta3)
        nc.vector.tensor_copy(b3[:, :, 12:14].bitcast(_F32R), a3[:, :, 14:16])  # qh, ql
        nc.vector.tensor_copy(b3[:, :, 14:16].bitcast(_F32R), ones3)

        # K-layout via fp32r PE transposes (lossless: every operand value is
        # 12-bit exact); the psum block for group g holds columns i = 4p+g in
        # p-order, un-permuted by the strided psum->SBUF operand copies
        # (rhs on ACT, lhsT on DVE).
        ops = []
        for b in range(BP):
            tpa = ptp.tile([K, N], _F32, tag="tpa")
            tpb = ptp.tile([K, N], _F32, tag="tpb")
            for g in range(NCH):
                i0 = (b * NCH + g) * K
                nc.tensor.transpose(
                    tpa[:, g * P : (g + 1) * P].bitcast(_F32R),
                    am[:, i0 : i0 + K].bitcast(_F32R),
                    identr,
                )
                nc.tensor.transpose(
                    tpb[:, g * P : (g + 1) * P].bitcast(_F32R),
                    bm[:, i0 : i0 + K].bitcast(_F32R),
                    identr,
                )
            rhs = mpool.tile([K, N], _F32R, tag="rhs")
            nc.scalar.copy(
                rhs[:].rearrange("k (p q) -> k q p", q=NCH),
                tpa[:].rearrange("k (q p) -> k q p", q=NCH),
            )
            lhsT = mpool.tile([K, N], _F32R, tag="lhsT")
            nc.vector.tensor_copy(
                lhsT[:].rearrange("k (p q) -> k q p", q=NCH),
                tpb[:].rearrange("k (q p) -> k q p", q=NCH),
            )
            ops.append((rhs, lhsT))

        # ---- per-batch compute: x+pe add + out store, then the arg matmuls
        # (fp32r), relu (DVE, psum f32 -> sbuf bf16), sqrt (ACT, bf16), and
        # half-tile ld stores so HBM writes start early.
        for b in range(BP):
            # x+pe add on DVE (bf16, ~1.05us) emitted between relu blocks:
            # it fills the DVE gap while this batch's matmuls run, and its
            # out store spreads write traffic over the window.
            xt = xts[b]
            nc.vector.tensor_add(xt[:], xt[:], pe_t[:])
            nc.sync.dma_start(
                out_o[b].rearrange("(p n) d -> p n d", n=NCH),
                xt[:].rearrange("p (n d) -> p n d", n=NCH),
            )

            rhs, lhsT = ops[b]
            ldt = ldpool.tile([P, NCH * N], _BF16)
            # last batch stores at chunk granularity to shorten the tail
            grain = 1 if b == BP - 1 else 2
            for n in range(NCH):
                argp = parg.tile([P, N], _F32)
                nc.tensor.matmul(
                    argp[:],
                    lhsT[:, n * P : (n + 1) * P],
                    rhs[:],
                    start=True,
                    stop=True,
                )
                nc.vector.tensor_scalar_max(
                    ldt[:, n * N : (n + 1) * N], argp[:], 0.0
                )
                if n % grain == grain - 1:
                    n0 = n - grain + 1
                    piece = ldt[:, n0 * N : (n + 1) * N]
                    nc.scalar.sqrt(piece, piece)
                    nc.sync.dma_start(
                        ld_o[b, n0 * P : (n + 1) * P].rearrange(
                            "(n p) j -> p n j", p=P
                        ),
                        piece.rearrange("p (n j) -> p n j", n=grain),
                    )

    nc.finalize()
    _cached_nc = nc
    return nc


def _run(x, x_coords, pe, trace=False):
    x = np.asarray(x)
    x_coords = np.ascontiguousarray(np.asarray(x_coords), dtype=np.float32)
    assert x.shape == (B, N, D) and x_coords.shape == (B, N, 4)
    xb = np.ascontiguousarray(x.astype(_BFNP))
    # pre-arrange coords: xc_arr[c][p, (b q k)] = x_coords[c*BP+b, 4p+q, k]
    xc_arr = np.ascontiguousarray(
        x_coords.reshape(NCORES, BP, P, NCH, 4).transpose(0, 2, 1, 3, 4)
    ).reshape(NCORES, P, BP * NCH * 4)

    nc = _build()
    in_maps = [
        {
            "x": xb[i * BP : (i + 1) * BP],
            "xc": xc_arr[i],
        }
        for i in range(NCORES)
    ]
    res = run_bass_kernel_spmd(nc, in_maps, list(range(NCORES)), trace=trace)
    out = np.concatenate(
        [np.asarray(res.results[i]["out"]) for i in range(NCORES)], axis=0
    ).astype(np.float32)
    ld = np.concatenate(
        [np.asarray(res.results[i]["ld"]) for i in range(NCORES)], axis=0
    ).astype(np.float32)
    return (out, ld), res


def kernel(x, x_coords, pe):
    (out, ld), _ = _run(x, x_coords, pe, trace=False)
    return (out, ld)
